# revision 10
# baseline (speedup 1.0000x reference)
"""Llama GQA attention (B=1, S=2048, DIM=4096, H=32, KVH=8, HD=128) on 8
Trainium2 cores, tensor-parallel over heads: each core owns 4 q-heads and
1 kv-head; Wo is sharded row-wise and partials are summed on the host.

Per-core dataflow (everything computed transposed so no activation
transposes are needed):
  QT[hd, s] = Wq_c^T @ hidden^T   (PSUM accum over 32 dim-chunks)
  KT likewise; V comes out as VT and is PE-transposed to [s, hd].
  RoPE on QT/KT in [hd, s] layout (halves on partition ranges 0:64/64:128).
  S^T[sk, sq] = K_i @ Q_h^T -> exp((S^T + mask)/sqrt(HD)) -> column sums via
  ones-matmul -> out^T[hd, sq] accumulated as V_i^T @ P^T, normalized by a
  broadcast-matmul of reciprocal sums.
  partial[s, 4096] = y^T chunks (lhsT) @ Wo_c, fused into the attention loop
  per sequence tile so projection/attention/output pipeline on the PE.

All tensors feeding the PE are float32r (rounded fp32): full PE rate at
free-dim 512 with ~1e-3 relative error end-to-end.
"""

import sys

sys.path.insert(0, "/opt/trn_rl_repo")

import numpy as np

import concourse.mybir as mybir
from concourse import bacc
from concourse.bass_utils import run_bass_kernel_spmd
from concourse.masks import make_identity
from concourse.tile import TileContext

S = 2048
DIM = 4096
HD = 128
NHL = 4  # q heads per core
SCALE = float(HD) ** 0.5
P = 128
SQT = 512  # sequence tile (free dim)
NSQ = S // SQT  # 4
KC = DIM // P  # 32 contraction chunks
NEG = -1e9

F32 = mybir.dt.float32
F32R = mybir.dt.float32r

_cache = {}


def build(mode, mm_dt=F32R, phases="ABC"):
    """mode in {'causal', 'zero', 'generic'}"""
    nc = bacc.Bacc("TRN2", target_bir_lowering=False, debug=False)
    DT = mm_dt  # dtype of everything that feeds the PE array

    ht = nc.declare_dram_parameter("ht", [DIM, S], DT, isOutput=False)
    wq = nc.declare_dram_parameter("wq", [DIM, NHL * HD], DT, isOutput=False)
    wk = nc.declare_dram_parameter("wk", [DIM, HD], DT, isOutput=False)
    wv = nc.declare_dram_parameter("wv", [DIM, HD], DT, isOutput=False)
    wo = nc.declare_dram_parameter("wo", [NHL * HD, DIM], DT, isOutput=False)
    cos2 = nc.declare_dram_parameter("cos2", [P, S], DT, isOutput=False)
    sinpm = nc.declare_dram_parameter("sinpm", [P, S], DT, isOutput=False)
    ones_kd = nc.declare_dram_parameter("ones_kd", [P, 1], DT, isOutput=False)
    swmd = nc.declare_dram_parameter("swmd", [P, P], DT, isOutput=False)
    ones_md = nc.declare_dram_parameter("ones_md", [1, P], DT, isOutput=False)
    if mode == "causal":
        bands = nc.declare_dram_parameter("bands", [4, P, SQT], F32, isOutput=False)
    if mode == "generic":
        maskt = nc.declare_dram_parameter("maskt", [S, S], F32, isOutput=False)
    out = nc.declare_dram_parameter("out", [S, DIM], F32, isOutput=True)

    ht_r = ht.rearrange("(ko p) s -> p ko s", p=P)  # [128, 32, 2048]
    wq_r = wq.rearrange("(ko p) n -> p ko n", p=P)  # [128, 32, 512]
    wk_r = wk.rearrange("(ko p) n -> p ko n", p=P)
    wv_r = wv.rearrange("(ko p) n -> p ko n", p=P)
    wo_r = wo.rearrange("(c p) n -> p c n", p=P)  # [128, 4, 4096]

    with TileContext(nc) as tc, nc.allow_low_precision(
        reason="float32r is rounded fp32; PSUM accumulation stays fp32"
    ):
        with (
            tc.tile_pool(name="const", bufs=1) as cpool,
            tc.tile_pool(name="persist", bufs=1) as ppool,
        ):
            ident = cpool.tile([P, P], F32)
            make_identity(nc, ident)
            ones_k = cpool.tile([P, 1], DT)
            nc.sync.dma_start(ones_k[:], ones_kd[:])
            ones_m = cpool.tile([1, P], DT)
            nc.sync.dma_start(ones_m[:], ones_md[:])
            swm = cpool.tile([P, P], DT)
            nc.sync.dma_start(swm[:], swmd[:])
            if mode == "causal":
                bsb = cpool.tile([P, 4, SQT], F32)
                nc.sync.dma_start(bsb[:], bands.rearrange("r p n -> p r n"))

            # per-j tiles so later phases depend only on the slices they read
            qt_t = [
                [ppool.tile([P, SQT], DT, name=f"qt{m}_{j}") for j in range(NSQ)]
                for m in range(NHL)
            ]
            kt_t = [ppool.tile([P, SQT], DT, name=f"kt{j}") for j in range(NSQ)]
            vn_t = [ppool.tile([P, 4, HD], DT, name=f"vn{j}") for j in range(NSQ)]

            # ---------------- Phase A: projections + RoPE -------------------
            with (
                tc.tile_pool(name="wA", bufs=1) as wpool,
                tc.tile_pool(name="htA", bufs=3) as htpool,
                tc.tile_pool(name="ropeA", bufs=1) as rpool,
                tc.tile_pool(name="cosA", bufs=2) as cpool2,
                tc.tile_pool(name="psA", bufs=1, space="PSUM") as psA,
                tc.tile_pool(name="psT", bufs=1, space="PSUM") as psT,
            ):
                # chunked weight loads so the first matmuls start early
                wq_t = wpool.tile([P, KC, NHL * HD], DT)
                wk_t = wpool.tile([P, KC, HD], DT)
                wv_t = wpool.tile([P, KC, HD], DT)
                for g in range(4):
                    ks = slice(8 * g, 8 * g + 8)
                    nc.sync.dma_start(wq_t[:, ks, :], wq_r[:, ks, :])
                    nc.gpsimd.dma_start(wk_t[:, ks, :], wk_r[:, ks, :])
                    nc.gpsimd.dma_start(wv_t[:, ks, :], wv_r[:, ks, :])

                for j in range(NSQ if "A" in phases else 0):
                    sq = slice(j * SQT, (j + 1) * SQT)
                    qps = [
                        psA.tile([P, SQT], F32, tag=f"q{m}", name=f"qps{m}")
                        for m in range(NHL)
                    ]
                    kps = psA.tile([P, SQT], F32, tag="k")
                    vps = psA.tile([P, SQT], F32, tag="v")
                    for q4 in range(8):  # 4-chunk groups of the contraction
                        htq = htpool.tile([P, 4, SQT], DT, tag="ht")
                        heng = nc.sync if q4 % 2 == 0 else nc.gpsimd
                        heng.dma_start(htq[:], ht_r[:, 4 * q4 : 4 * q4 + 4, sq])
                        for c in range(4):
                            kc = 4 * q4 + c
                            st, sp = kc == 0, kc == KC - 1
                            rhs = htq[:, c, :]
                            for m in range(NHL):
                                nc.tensor.matmul(
                                    qps[m],
                                    wq_t[:, kc, m * HD : (m + 1) * HD],
                                    rhs,
                                    start=st,
                                    stop=sp,
                                )
                            nc.tensor.matmul(
                                kps, wk_t[:, kc, :], rhs, start=st, stop=sp
                            )
                            nc.tensor.matmul(
                                vps, wv_t[:, kc, :], rhs, start=st, stop=sp
                            )

                    # V: copy VT slice then PE-transpose to natural layout
                    vtt = rpool.tile([P, SQT], F32, tag="vtt")
                    nc.vector.tensor_copy(out=vtt[:], in_=vps)
                    for u in range(SQT // P):
                        tps = psT.tile([P, P], F32, tag="tp")
                        nc.tensor.transpose(tps, vtt[:, u * P : (u + 1) * P], ident)
                        nc.vector.tensor_copy(out=vn_t[j][:, u, :], in_=tps)

                    cosj = cpool2.tile([P, SQT], DT, tag="cos")
                    nc.sync.dma_start(cosj[:], cos2[:, sq])
                    sinj = cpool2.tile([P, SQT], DT, tag="sin")
                    nc.sync.dma_start(sinj[:], sinpm[:, sq])

                    for idx, ps in enumerate(qps + [kps]):
                        tile_ = qt_t[idx][j] if idx < NHL else kt_t[j]
                        dst = tile_[:]
                        nc.vector.tensor_copy(out=dst, in_=ps)
                        swp = psT.tile([P, SQT], F32, tag="swp")
                        nc.tensor.matmul(swp, swm[:], dst, start=True, stop=True)
                        t1 = rpool.tile([P, SQT], DT, tag="t1")
                        nc.vector.tensor_mul(out=t1[:], in0=dst, in1=cosj[:])
                        t2 = rpool.tile([P, SQT], DT, tag="t2")
                        nc.vector.tensor_mul(out=t2[:], in0=swp, in1=sinj[:])
                        nc.vector.tensor_add(out=dst, in0=t1[:], in1=t2[:])

            # ------------- Phase B+C: attention fused with out-proj ---------
            with (
                tc.tile_pool(name="woBC", bufs=1) as wopool,
                tc.tile_pool(name="stB", bufs=6) as stpool,
                tc.tile_pool(name="mB", bufs=2) as mpool,
                tc.tile_pool(name="smB", bufs=2) as smpool,
                tc.tile_pool(name="ytBC", bufs=1) as ytpool,
                tc.tile_pool(name="oC", bufs=3) as opool,
                tc.tile_pool(name="psS", bufs=4, space="PSUM") as psS,
                tc.tile_pool(name="psO", bufs=1, space="PSUM") as psO,
                tc.tile_pool(name="psR", bufs=1, space="PSUM") as psR,
                tc.tile_pool(name="psC", bufs=2, space="PSUM") as psC,
            ):
                wo_t = wopool.tile([P, NHL, DIM], DT)  # resident Wo shard
                for g in range(4):
                    nc.gpsimd.dma_start(
                        wo_t[:, :, g * 1024 : (g + 1) * 1024],
                        wo_r[:, :, g * 1024 : (g + 1) * 1024],
                    )

                for j in range(NSQ if "B" in phases else 0):
                    sq = slice(j * SQT, (j + 1) * SQT)
                    n_act = 4 * j + 4 if mode == "causal" else S // P
                    if mode == "generic":
                        mtj = mpool.tile([P, S // P, SQT], F32, tag="mt")
                        nc.sync.dma_start(
                            mtj[:], maskt.rearrange("(i p) q -> p i q", p=P)[:, :, sq]
                        )
                    ytj = ytpool.tile([P, NHL, SQT], DT, name=f"yt{j}")
                    for h in range(NHL):
                        ops = psO.tile([P, SQT], F32, tag="o")
                        sums = psR.tile([1, SQT], F32, tag="s")
                        for i in range(n_act):
                            sps = psS.tile([P, SQT], F32, tag="sc")
                            nc.tensor.matmul(
                                sps,
                                kt_t[i // 4][:, (i % 4) * P : (i % 4 + 1) * P],
                                qt_t[h][j][:],
                                start=True,
                                stop=True,
                            )
                            if mode == "causal" and i >= 4 * j:
                                nc.vector.tensor_add(
                                    out=sps, in0=sps, in1=bsb[:, i - 4 * j, :]
                                )
                            elif mode == "generic":
                                nc.vector.tensor_add(out=sps, in0=sps, in1=mtj[:, i, :])
                            stt = stpool.tile([P, SQT], DT, tag="st")
                            nc.scalar.activation(
                                stt[:], sps, mybir.ActivationFunctionType.Exp,
                                scale=1.0 / SCALE,
                            )
                            st, sp = i == 0, i == n_act - 1
                            nc.tensor.matmul(
                                sums, ones_k[:], stt[:], start=st, stop=sp
                            )
                            nc.tensor.matmul(
                                ops, vn_t[i // 4][:, i % 4, :], stt[:], start=st, stop=sp
                            )
                        rec = smpool.tile([1, SQT], DT, tag="rec")
                        nc.vector.reciprocal(out=rec[:], in_=sums)
                        bps = psS.tile([P, SQT], F32, tag="sc")
                        nc.tensor.matmul(
                            bps, ones_m[:], rec[:], start=True, stop=True
                        )
                        bsb2 = smpool.tile([P, SQT], F32, tag="bc")
                        nc.vector.tensor_copy(out=bsb2[:], in_=bps)
                        nc.vector.tensor_mul(out=ytj[:, h, :], in0=ops, in1=bsb2[:])

                    # out-projection for this sequence tile (rows 512j..512j+511)
                    if "C" in phases:
                        for u in range(4):
                            ug = 4 * j + u
                            for n in range(DIM // SQT):
                                nn = slice(n * SQT, (n + 1) * SQT)
                                cps = psC.tile([P, SQT], F32, tag="c")
                                for c in range(NHL):
                                    nc.tensor.matmul(
                                        cps,
                                        ytj[:, c, u * P : (u + 1) * P],
                                        wo_t[:, c, nn],
                                        start=c == 0,
                                        stop=c == NHL - 1,
                                    )
                                osb = opool.tile([P, SQT], F32, tag="ob")
                                nc.any.tensor_copy(out=osb[:], in_=cps)
                                eng = nc.sync if n % 2 == 0 else nc.gpsimd
                                eng.dma_start(out[ug * P : (ug + 1) * P, nn], osb[:])

    nc.compile()
    return nc


def _get(mode, mm_dt):
    key = (mode, str(mm_dt))
    if key not in _cache:
        _cache[key] = build(mode, mm_dt)
    return _cache[key]


def kernel(hidden_states, freqs_cos, freqs_sin, atten_mask, Wq, Wk, Wv, Wo,
           mm_dt=F32R, _return_raw=False):
    hidden_states = np.asarray(hidden_states)
    m = np.asarray(atten_mask)[0, 0]
    r = np.arange(S)
    vis = r[None, :] <= r[:, None]  # [q, k] visible
    if np.all(m[vis] == 0) and np.all(m[~vis] <= -1e8):
        mode = "causal"
    elif not m.any():
        mode = "zero"
    else:
        mode = "generic"

    ht = np.ascontiguousarray(hidden_states[0].T, dtype=np.float32)
    ct = np.asarray(freqs_cos, np.float32).T
    st = np.asarray(freqs_sin, np.float32).T
    cos2 = np.ascontiguousarray(np.concatenate([ct, ct], 0))
    sinpm = np.ascontiguousarray(np.concatenate([-st, st], 0))

    in_common = {"ht": ht, "cos2": cos2, "sinpm": sinpm,
                 "ones_kd": np.ones((P, 1), np.float32),
                 "ones_md": np.ones((1, P), np.float32),
                 "swmd": np.eye(P, k=64, dtype=np.float32)
                         + np.eye(P, k=-64, dtype=np.float32)}
    if mode == "causal":
        ii = np.arange(P)[:, None]
        jj = np.arange(SQT)[None, :]
        in_common["bands"] = np.stack(
            [np.where(jj < ii + 128 * t, np.float32(NEG), np.float32(0))
             for t in range(4)]
        ).astype(np.float32)
    elif mode == "generic":
        in_common["maskt"] = np.ascontiguousarray(m.T * np.float32(SCALE), np.float32)

    Wq = np.asarray(Wq, np.float32)
    Wk = np.asarray(Wk, np.float32)
    Wv = np.asarray(Wv, np.float32)
    Wo = np.asarray(Wo, np.float32)
    in_maps = []
    for c in range(8):
        in_maps.append(
            dict(
                in_common,
                wq=np.ascontiguousarray(Wq[:, c * 512 : (c + 1) * 512]),
                wk=np.ascontiguousarray(Wk[:, c * 128 : (c + 1) * 128]),
                wv=np.ascontiguousarray(Wv[:, c * 128 : (c + 1) * 128]),
                wo=np.ascontiguousarray(Wo[c * 512 : (c + 1) * 512, :]),
            )
        )

    nc = _get(mode, mm_dt)
    res = run_bass_kernel_spmd(nc, in_maps, list(range(8)))
    if _return_raw:
        return res
    partials = np.stack([res.results[c]["out"] for c in range(8)])
    return partials.sum(0, dtype=np.float32)[None]
